# revision 1
# baseline (speedup 1.0000x reference)
"""Multi-head attention with RoPE on 8 Trainium2 NeuronCores.

Sharding: core c handles batch b = c//2 and head-group hg = c%2 (8 of 16
heads).  Data-parallel over batch, tensor-parallel over heads; the
row-parallel wo all-reduce (2 cores per batch) happens on the host during
the gather/unshard step.

Per-core program (single SPMD NEFF, no collectives):
  1. QT = wq_p @ x^T, KT = wk_p @ x^T   ([512, 2048], head-dim on partitions)
     V  = x @ wv_p^T                     ([2048, 520] with a ones column per
                                          head -> softmax denominator for free)
  2. RoPE on QT/KT.  Host permutes wq/wk rows per head to [evens | odds], so
     the rotation becomes dense [128, 2048] elementwise ops with precomputed
     cos/sin tables; the pair-swap is a partition-block swap done by
     SBUF->SBUF DMA.
  3. Causal attention per head, scores^T [k, q] blocks:
     exp((K Q^T)/8) on ACT (scores are provably < ~4 here, so no
     max-subtraction), tri-mask on the diagonal 128-blocks, then
     attn^T.T @ V_aug accumulated in PSUM -> [65, q]: rows 0:64 the output,
     row 64 the denominator.  Normalize with a PE ones-outer-product
     broadcast of 1/denom.
  4. out^T_partial = wo_p @ A  ([1024, 2048]) -> DMA out; host sums the two
     head-group partials and transposes.
"""

import sys
import types

sys.path.insert(0, "/opt/trn_rl_repo")

import numpy as np

import concourse.bacc as bacc
import concourse.mybir as mybir
import concourse.tile as tile
from concourse.bass_utils import run_bass_kernel_spmd

# Problem constants (hardcoded per contract)
B, S, D = 4, 2048, 1024
H = 16
DH = D // H          # 64
THETA = 10000.0
NCORES = 8
HG = 2               # head groups (tensor-parallel factor)
HD = D // HG         # 512 = per-core heads dim
NH = H // HG         # 8 heads per core
P = 128
SC = 512             # q-chunk
NSC = S // SC        # 4
NKB = S // P         # 16 k-blocks
NDB = D // P         # 8 d-blocks (contraction for projections)
SCALE = 1.0 / np.sqrt(np.float32(DH))

F32 = mybir.dt.float32
F32R = mybir.dt.float32r  # PE streams 1 row/cycle for N>=256 (vs 4 for f32)


def _install_ntff_hook():
    """Best-effort: register the axon NTFF profile hook so trace=True works."""
    try:
        import antenv

        if "antenv.axon_hooks" in sys.modules:
            return
        sys.path.insert(0, "/root/.axon_site/trn_agent_boot")
        import trn_boot

        hook = trn_boot._ntff_profile_via_ctypes("/opt/axon/libaxon_pjrt.so")
        mod = types.ModuleType("antenv.axon_hooks")
        mod.get_axon_ntff_profile_hook = lambda: hook
        mod.set_axon_ntff_profile_hook = lambda h: None
        sys.modules["antenv.axon_hooks"] = mod
        antenv.axon_hooks = mod
    except Exception:
        pass


def build_program(phase="full"):
    nc = bacc.Bacc("TRN2", target_bir_lowering=False, debug=False,
                   num_devices=NCORES)

    xt_d = nc.dram_tensor("xt", [D, S], F32R, kind="ExternalInput")
    wqt_d = nc.dram_tensor("wqt", [D, HD], F32R, kind="ExternalInput")
    wkt_d = nc.dram_tensor("wkt", [D, HD], F32R, kind="ExternalInput")
    wvt_d = nc.dram_tensor("wvt", [D, HD], F32R, kind="ExternalInput")
    wot_d = nc.dram_tensor("wot", [HD, D], F32R, kind="ExternalInput")
    cf_d = nc.dram_tensor("cfull", [P, S], F32R, kind="ExternalInput")
    sf_d = nc.dram_tensor("sfull", [P, S], F32R, kind="ExternalInput")
    tri_d = nc.dram_tensor("tri", [P, P], F32R, kind="ExternalInput")
    ones_d = nc.dram_tensor("ones", [P, NH], F32R, kind="ExternalInput")
    out_d = nc.dram_tensor("outT", [D, S], F32, kind="ExternalOutput")

    EXP = mybir.ActivationFunctionType.Exp
    MULT = mybir.AluOpType.mult
    ADD = mybir.AluOpType.add

    with tile.TileContext(nc) as tc:
        with (
            tc.tile_pool(name="big", bufs=8) as big,          # [128,2048] slots
            tc.tile_pool(name="qk", bufs=8) as qk,            # QT+KT persistent
            tc.tile_pool(name="vg", bufs=NKB) as vgp,         # V_aug persistent
            tc.tile_pool(name="w", bufs=10) as wp,            # [128,512] slots
            tc.tile_pool(name="wo", bufs=4) as wop,
            tc.tile_pool(name="small", bufs=4) as small,
            tc.tile_pool(name="psA", bufs=2, space="PSUM") as psA,   # proj + wo
            tc.tile_pool(name="psS", bufs=2, space="PSUM") as psS,   # scores
            tc.tile_pool(name="psV", bufs=2, space="PSUM") as psV,   # attn@V
            tc.tile_pool(name="psB", bufs=1, space="PSUM") as psB,   # bcast
        ):
            # ---- load x^T and the tri mask ----
            xt = []
            for k in range(NDB):
                t = big.tile([P, S], F32R, tag="big", name=f"xt{k}")
                nc.sync.dma_start(t[:], xt_d[P * k:P * (k + 1), :])
                xt.append(t)
            tri = small.tile([P, P], F32R, tag="tri", bufs=1)
            nc.sync.dma_start(tri[:], tri_d[:])

            # ---- projections ----
            def load_w(dram, tag):
                ts_ = []
                for k in range(NDB):
                    t = wp.tile([P, HD], F32R, tag="w", name=f"w{k}")
                    nc.sync.dma_start(t[:], dram[P * k:P * (k + 1), :])
                    ts_.append(t)
                return ts_

            def project_T(wt, out_tiles):
                # out[m][:, n*SC:] = sum_k wt[k][:, m*128:+128].T @ xt[k][:, n]
                for m in range(HD // P):
                    for n in range(NSC):
                        ps = psA.tile([P, SC], F32, tag="psA")
                        for k in range(NDB):
                            nc.tensor.matmul(
                                ps[:],
                                (wt[k][:, P * m:P * (m + 1)]),
                                (xt[k][:, SC * n:SC * (n + 1)]),
                                start=(k == 0), stop=(k == NDB - 1),
                            )
                        nc.scalar.copy(out_tiles[m][:, SC * n:SC * (n + 1)], ps[:])

            wq_t = load_w(wqt_d, "w")
            QT = [qk.tile([P, S], F32R, tag=f"qt{m}", bufs=1, name=f"qt{m}")
                  for m in range(HD // P)]
            project_T(wq_t, QT)

            wk_t = load_w(wkt_d, "w")
            KT = [qk.tile([P, S], F32R, tag=f"kt{m}", bufs=1, name=f"kt{m}")
                  for m in range(HD // P)]
            project_T(wk_t, KT)

            # V in [s, heads*65] layout with ones column per head
            wv_t = load_w(wvt_d, "w")
            Vg = []
            for j in range(NKB):
                vt = vgp.tile([P, NH * (DH + 1)], F32R, tag=f"vg{j}", bufs=1,
                              name=f"vg{j}")
                v3 = vt[:].rearrange("p (h c) -> p h c", h=NH)
                nc.sync.dma_start(v3[:, :, DH:DH + 1], ones_d[:, :, None])
                ps = psA.tile([P, HD], F32, tag="psA")
                for k in range(NDB):
                    nc.tensor.matmul(
                        ps[:],
                        (xt[k][:, P * j:P * (j + 1)]),
                        (wv_t[k][:]),
                        start=(k == 0), stop=(k == NDB - 1),
                    )
                nc.vector.tensor_copy(
                    v3[:, :, 0:DH], ps[:].rearrange("p (h c) -> p h c", h=NH))
                Vg.append(vt)

            if phase == "proj":
                for m in range(HD // P):
                    for n in range(NSC):
                        ot0 = wp.tile([P, SC], F32, tag="w", name="dbg0")
                        nc.scalar.copy(ot0[:], QT[m][:, SC * n:SC * (n + 1)])
                        nc.sync.dma_start(out_d[P * m:P * (m + 1), SC * n:SC * (n + 1)], ot0[:])
                        ot1 = wp.tile([P, SC], F32, tag="w", name="dbg1")
                        nc.scalar.copy(ot1[:], KT[m][:, SC * n:SC * (n + 1)])
                        nc.sync.dma_start(out_d[HD + P * m:HD + P * (m + 1), SC * n:SC * (n + 1)], ot1[:])
            # ---- RoPE on QT and KT ----
            # rows of each 128-tile: [h0.x1 | h0.x2 | h1.x1 | h1.x2] (32 each)
            # out = t * cfull + swap32pairs(t) * sfull
            if phase != "proj":
                cf = big.tile([P, S], F32R, tag="big", name="cf")
                nc.sync.dma_start(cf[:], cf_d[:])
                sf = big.tile([P, S], F32R, tag="big", name="sf")
                nc.sync.dma_start(sf[:], sf_d[:])
            for t in (QT + KT if phase != "proj" else []):
                sw = big.tile([P, S], F32R, tag="big", name="sw")
                for g in range(4):
                    src = (g ^ 1) * 32
                    nc.sync.dma_start(sw[g * 32:(g + 1) * 32, :],
                                      t[src:src + 32, :])
                nc.vector.tensor_tensor(t[:], t[:], cf[:], MULT)
                nc.gpsimd.tensor_tensor(sw[:], sw[:], sf[:], MULT)
                nc.vector.tensor_tensor(t[:], t[:], sw[:], ADD)

            if phase == "rope":
                for m in range(HD // P):
                    for n in range(NSC):
                        ot0 = wp.tile([P, SC], F32, tag="w", name="dbg0")
                        nc.scalar.copy(ot0[:], QT[m][:, SC * n:SC * (n + 1)])
                        nc.sync.dma_start(out_d[P * m:P * (m + 1), SC * n:SC * (n + 1)], ot0[:])
                        ot1 = wp.tile([P, SC], F32, tag="w", name="dbg1")
                        nc.scalar.copy(ot1[:], KT[m][:, SC * n:SC * (n + 1)])
                        nc.sync.dma_start(out_d[HD + P * m:HD + P * (m + 1), SC * n:SC * (n + 1)], ot1[:])
            # ---- attention ----
            A = [big.tile([P, S], F32R, tag="big", name=f"a{m}") for m in range(HD // P)]
            import os as _os
            _hlist = range(NH) if phase in ("full", "attn") else []
            if _os.environ.get("MHA_HEADS"):
                _hlist = [int(v) for v in _os.environ["MHA_HEADS"].split(",")]
            _qclist = range(NSC)
            if _os.environ.get("MHA_QCS"):
                _qclist = [int(v) for v in _os.environ["MHA_QCS"].split(",")]
            for h in _hlist:
                ht, ho = divmod(h, 2)
                ho *= DH
                for qc in _qclist:
                    nj = 4 * qc + 4
                    av = psV.tile([DH + 1, SC], F32, tag="psV")
                    for j in range(nj):
                        d = j - 4 * qc
                        q0 = P * d if d >= 0 else 0
                        sc_ps = psS.tile([P, SC], F32, tag="psS")
                        nc.tensor.matmul(
                            sc_ps[:, q0:SC],
                            (KT[ht][ho:ho + DH, P * j:P * (j + 1)]),
                            (QT[ht][ho:ho + DH, SC * qc + q0:SC * (qc + 1)]),
                            start=True, stop=True,
                        )
                        at = wp.tile([P, SC], F32R, tag="w")
                        nc.scalar.activation(at[:, q0:SC], sc_ps[:, q0:SC],
                                             EXP, scale=float(SCALE))
                        if d >= 0:
                            nc.vector.tensor_tensor(
                                at[:, q0:q0 + P], at[:, q0:q0 + P], tri[:],
                                MULT)
                        nc.tensor.matmul(
                            av[:, q0:SC],
                            (Vg[j][:, (DH + 1) * h:(DH + 1) * (h + 1)]),
                            (at[:, q0:SC]),
                            start=(j == 0), stop=(j == nj - 1),
                        )
                    rr = small.tile([1, SC], F32R, tag="rr", bufs=2, name="rr")
                    with nc.allow_low_precision(reason="f32r matmul feed"):
                        nc.vector.reciprocal(rr[:], av[DH:DH + 1, :])
                    bc = psB.tile([DH, SC], F32, tag="psB")
                    nc.tensor.matmul(bc[:], (tri[0:1, 0:DH]), (rr[:]),
                                     start=True, stop=True)
                    rb = small.tile([DH, SC], F32, tag="rb", bufs=2, name="rb")
                    nc.vector.tensor_copy(rb[:], bc[:])
                    nc.vector.tensor_tensor(
                        A[ht][ho:ho + DH, SC * qc:SC * (qc + 1)],
                        av[0:DH, :], rb[:], MULT)

            if phase == "attn":
                for m in sorted({h // 2 for h in _hlist}):
                    for n in range(NSC):
                        ot0 = wp.tile([P, SC], F32, tag="w", name="dbg0")
                        nc.scalar.copy(ot0[:], A[m][:, SC * n:SC * (n + 1)])
                        nc.sync.dma_start(
                            out_d[P * m:P * (m + 1), SC * n:SC * (n + 1)], ot0[:])
            # ---- output projection (row-parallel partial) ----
            wo_t = []
            for k in (range(HD // P) if phase == "full" else []):
                t = wop.tile([P, D], F32R, tag="wot", name=f"wo{k}")
                nc.sync.dma_start(t[:], wot_d[P * k:P * (k + 1), :])
                wo_t.append(t)
            for m in (range(D // P) if phase == "full" else []):
                for n in range(NSC):
                    ps = psA.tile([P, SC], F32, tag="psA")
                    for k in range(HD // P):
                        nc.tensor.matmul(
                            ps[:],
                            (wo_t[k][:, P * m:P * (m + 1)]),
                            (A[k][:, SC * n:SC * (n + 1)]),
                            start=(k == 0), stop=(k == HD // P - 1),
                        )
                    ot = wp.tile([P, SC], F32, tag="w")
                    nc.scalar.copy(ot[:], ps[:])
                    nc.sync.dma_start(
                        out_d[P * m:P * (m + 1), SC * n:SC * (n + 1)], ot[:])

    nc.compile()
    return nc


_NC_CACHE = []


def _get_nc():
    if not _NC_CACHE:
        _NC_CACHE.append(build_program())
    return _NC_CACHE[0]


def _host_tables(token_positions):
    pos = np.asarray(token_positions).astype(np.float32)
    inv_freq = np.float32(THETA) ** (
        -np.arange(0, DH, 2, dtype=np.float32) / np.float32(DH))
    ang = pos[:, None] * inv_freq[None, :]            # [S, 32] f32
    cos_t = np.ascontiguousarray(np.cos(ang).T)        # [32, S]
    sin_t = np.ascontiguousarray(np.sin(ang).T)
    cfull = np.tile(cos_t, (4, 1)).astype(np.float32)  # [128, S]
    sfull = np.concatenate([-sin_t, sin_t, -sin_t, sin_t], 0).astype(np.float32)
    return cfull, sfull


def kernel(in_features, token_positions, wq, wk, wv, wo):
    _install_ntff_hook()
    x = np.asarray(in_features, dtype=np.float32)
    wq = np.asarray(wq, dtype=np.float32)
    wk = np.asarray(wk, dtype=np.float32)
    wv = np.asarray(wv, dtype=np.float32)
    wo = np.asarray(wo, dtype=np.float32)

    cfull, sfull = _host_tables(token_positions)
    tri = np.triu(np.ones((P, P), dtype=np.float32))   # keep k_row <= q_col
    ones = np.ones((P, NH), dtype=np.float32)

    # per-head row permutation: evens then odds
    perm1 = np.concatenate([np.arange(0, DH, 2), np.arange(1, DH, 2)])
    perm = np.concatenate([h * DH + perm1 for h in range(NH)])

    in_maps = []
    for c in range(NCORES):
        b, hg = divmod(c, HG)
        sl = slice(hg * HD, (hg + 1) * HD)
        in_maps.append({
            "xt": np.ascontiguousarray(x[b].T),
            "wqt": np.ascontiguousarray(wq[sl][perm].T),
            "wkt": np.ascontiguousarray(wk[sl][perm].T),
            "wvt": np.ascontiguousarray(wv[sl].T),
            "wot": np.ascontiguousarray(wo[:, sl].T),
            "cfull": cfull,
            "sfull": sfull,
            "tri": tri,
            "ones": ones,
        })

    nc = _get_nc()
    res = run_bass_kernel_spmd(nc, in_maps, list(range(NCORES)))

    out = np.empty((B, S, D), dtype=np.float32)
    for b in range(B):
        acc = res.results[2 * b]["outT"] + res.results[2 * b + 1]["outT"]
        out[b] = acc.T
    return out



# revision 7
# speedup vs baseline: 1.3889x; 1.3889x over previous
"""Multi-head attention with RoPE on 8 Trainium2 NeuronCores.

Sharding: core c handles batch b = c//2 and head-group hg = c%2 (8 of 16
heads).  Data-parallel over batch, tensor-parallel over heads; the
row-parallel wo all-reduce (2 cores per batch) happens on the host during
the gather/unshard step.

v2 per-core program (single SPMD NEFF, no collectives):
  - bf16 storage for x^T, weights, Q/K/V, attention weights and output
    (psum accumulation stays f32); halves DMA and SBUF traffic.
  - Projections: QT/KT = w @ x^T with 8-step psum accumulation; V in
    [s, 8*(64+1)] layout with a ones column per head (softmax denominator
    rides the attn@V matmul for free).
  - RoPE: host permutes wq/wk rows per head so the pair-swap is a
    16<->16 swap inside each 32-partition quadrant -> one DVE
    stream_shuffle (no SBUF<->SBUF DMA), then mul/mul/add with
    precomputed cos/sin tables (DVE + gpsimd).
  - Attention per (head-pair, q-chunk): for each 128-k-block j, the two
    heads' scores^T go into one 2-bank psum tile [128, 1024] via two
    row-tiled (64-contraction) matmuls that run concurrently on the PE;
    one exp ACTIVATE covers both heads (halves ACT instruction
    overhead); causal tri-mask on gpsimd; attn@V accumulates [65, 512]
    per head in psum.  Normalize via reciprocal (DVE) + ones-outer
    broadcast matmul (PE) + multiply (DVE).
  - wo projection per q-chunk is interleaved right after each q-chunk's
    attention so output DMA overlaps the next chunk's compute.
"""

import sys
import types

sys.path.insert(0, "/opt/trn_rl_repo")

import numpy as np

import concourse.bacc as bacc
import concourse.mybir as mybir
import concourse.tile as tile
from concourse.bass_utils import run_bass_kernel_spmd

# Problem constants (hardcoded per contract)
B, S, D = 4, 2048, 1024
H = 16
DH = D // H          # 64
THETA = 10000.0
NCORES = 8
HG = 2               # head groups (tensor-parallel factor)
HD = D // HG         # 512 = per-core heads dim
NH = H // HG         # 8 heads per core
P = 128
SC = 512             # q-chunk
NSC = S // SC        # 4
NKB = S // P         # 16 k-blocks
NDB = D // P         # 8 d-blocks (contraction for projections)
SCALE = 1.0 / np.sqrt(np.float32(DH))

F32 = mybir.dt.float32
F32R = mybir.dt.float32r
BF16 = mybir.dt.bfloat16

# stream_shuffle: swap halves within each 32-partition quadrant
SHUF_MASK = list(range(16, 32)) + list(range(16))


def _install_ntff_hook():
    """Best-effort: register the axon NTFF profile hook so trace=True works."""
    try:
        import antenv

        if "antenv.axon_hooks" in sys.modules:
            return
        sys.path.insert(0, "/root/.axon_site/trn_agent_boot")
        import trn_boot

        hook = trn_boot._ntff_profile_via_ctypes("/opt/axon/libaxon_pjrt.so")
        mod = types.ModuleType("antenv.axon_hooks")
        mod.get_axon_ntff_profile_hook = lambda: hook
        mod.set_axon_ntff_profile_hook = lambda h: None
        sys.modules["antenv.axon_hooks"] = mod
        antenv.axon_hooks = mod
    except Exception:
        pass


def build_program():
    nc = bacc.Bacc("TRN2", target_bir_lowering=False, debug=False,
                   num_devices=NCORES)

    xt_d = nc.dram_tensor("xt", [D, S], BF16, kind="ExternalInput")
    wqt_d = nc.dram_tensor("wqt", [D, HD], BF16, kind="ExternalInput")
    wkt_d = nc.dram_tensor("wkt", [D, HD], BF16, kind="ExternalInput")
    wvt_d = nc.dram_tensor("wvt", [D, HD], BF16, kind="ExternalInput")
    wot_d = nc.dram_tensor("wot", [HD, D], BF16, kind="ExternalInput")
    cf_d = nc.dram_tensor("cfull", [P, S], BF16, kind="ExternalInput")
    sf_d = nc.dram_tensor("sfull", [P, S], BF16, kind="ExternalInput")
    tri_d = nc.dram_tensor("tri", [P, P], BF16, kind="ExternalInput")
    onesr_d = nc.dram_tensor("onesr", [1, DH], F32R, kind="ExternalInput")
    onesb_d = nc.dram_tensor("onesb", [P, NH], BF16, kind="ExternalInput")
    negc_d = nc.dram_tensor("negc", [P, 3 * P], BF16, kind="ExternalInput")
    out_d = nc.dram_tensor("outT", [D, S], BF16, kind="ExternalOutput")

    EXP = mybir.ActivationFunctionType.Exp
    MULT = mybir.AluOpType.mult
    ADD = mybir.AluOpType.add

    with tile.TileContext(nc) as tc:
        with (
            tc.tile_pool(name="xtp", bufs=NDB) as xtp,
            tc.tile_pool(name="qk", bufs=8) as qk,
            tc.tile_pool(name="vg", bufs=NKB) as vgp,
            tc.tile_pool(name="w", bufs=8) as wp,
            tc.tile_pool(name="wo", bufs=4) as wop,
            tc.tile_pool(name="ap", bufs=4) as ap_pool,
            tc.tile_pool(name="at2", bufs=3) as at2p,
            tc.tile_pool(name="sw", bufs=2) as swp,
            tc.tile_pool(name="small", bufs=8) as small,
            tc.tile_pool(name="ot", bufs=4) as otp,
            tc.tile_pool(name="psA", bufs=2, space="PSUM") as psA,
            tc.tile_pool(name="psS", bufs=2, space="PSUM") as psS,
            tc.tile_pool(name="psV", bufs=2, space="PSUM") as psV,
        ):
            # ---- constants / DMAs (weights first so Q-proj starts early) ----
            ones64 = small.tile([1, DH], F32R, tag="ones64", bufs=1)
            nc.sync.dma_start(ones64[:], onesr_d[:])
            ones_sb = small.tile([P, NH], BF16, tag="onesb", bufs=1)
            nc.sync.dma_start(ones_sb[:], onesb_d[:])
            negt = small.tile([P, 3 * P], BF16, tag="negc", bufs=1)
            nc.sync.dma_start(negt[:], negc_d[:])

            wq_t = []
            for k in range(NDB):
                t = wp.tile([P, HD], BF16, tag="w", name=f"wq{k}")
                nc.sync.dma_start(t[:], wqt_d[P * k:P * (k + 1), :])
                wq_t.append(t)
            xt = []
            for k in range(NDB):
                t = xtp.tile([P, S], BF16, tag="xt", name=f"xt{k}")
                nc.sync.dma_start(t[:], xt_d[P * k:P * (k + 1), :])
                xt.append(t)
            wk_t = []
            for k in range(NDB):
                t = wp.tile([P, HD], BF16, tag="w", name=f"wk{k}")
                nc.sync.dma_start(t[:], wkt_d[P * k:P * (k + 1), :])
                wk_t.append(t)
            wv_t = []
            for k in range(NDB):
                t = wp.tile([P, HD], BF16, tag="w", name=f"wv{k}")
                nc.sync.dma_start(t[:], wvt_d[P * k:P * (k + 1), :])
                wv_t.append(t)
            cf = small.tile([P, S], BF16, tag="cf", bufs=1)
            nc.sync.dma_start(cf[:], cf_d[:])
            sf = small.tile([P, S], BF16, tag="sf", bufs=1)
            nc.sync.dma_start(sf[:], sf_d[:])
            tri = small.tile([P, P], BF16, tag="tri", bufs=1)
            nc.sync.dma_start(tri[:], tri_d[:])
            wo_t = []
            for k in range(HD // P):
                t = wop.tile([P, D], BF16, tag="wot", name=f"wo{k}")
                nc.sync.dma_start(t[:], wot_d[P * k:P * (k + 1), :])
                wo_t.append(t)

            # ---- Q/K projections + RoPE ----
            def rope(t):
                sw = swp.tile([P, S], BF16, tag="sw", name="sw")
                nc.vector.stream_shuffle(sw[:], t[:], SHUF_MASK)
                nc.gpsimd.tensor_tensor(sw[:], sw[:], sf[:], MULT)
                nc.vector.tensor_tensor(t[:], t[:], cf[:], MULT)
                nc.vector.tensor_tensor(t[:], t[:], sw[:], ADD)

            def project_T(wt, out_tiles, do_rope):
                for m in range(HD // P):
                    for n in range(NSC):
                        ps = psA.tile([P, SC], F32, tag="psA")
                        for k in range(NDB):
                            nc.tensor.matmul(
                                ps[:],
                                (wt[k][:, P * m:P * (m + 1)]),
                                (xt[k][:, SC * n:SC * (n + 1)]),
                                start=(k == 0), stop=(k == NDB - 1),
                            )
                        nc.vector.tensor_copy(
                            out_tiles[m][:, SC * n:SC * (n + 1)], ps[:])
                    if do_rope:
                        rope(out_tiles[m])

            QT = [qk.tile([P, S], BF16, tag=f"qt{m}", bufs=1, name=f"qt{m}")
                  for m in range(HD // P)]
            project_T(wq_t, QT, True)
            KT = [qk.tile([P, S], BF16, tag=f"kt{m}", bufs=1, name=f"kt{m}")
                  for m in range(HD // P)]
            project_T(wk_t, KT, True)

            # ---- V projection (j = k-block); interleaved with attention ----
            Vg = [None] * NKB

            def vproj(j):
                vt = vgp.tile([P, NH * (DH + 1)], BF16, tag=f"vg{j}", bufs=1,
                              name=f"vg{j}")
                v3 = vt[:].rearrange("p (h c) -> p h c", h=NH)
                ps = psA.tile([P, HD], F32, tag="psA")
                for k in range(NDB):
                    nc.tensor.matmul(
                        ps[:],
                        (xt[k][:, P * j:P * (j + 1)]),
                        (wv_t[k][:]),
                        start=(k == 0), stop=(k == NDB - 1),
                    )
                nc.vector.tensor_copy(
                    v3[:, :, 0:DH], ps[:].rearrange("p (h c) -> p h c", h=NH))
                nc.vector.tensor_copy(v3[:, :, DH:DH + 1], ones_sb[:, :, None])
                Vg[j] = vt

            for j in range(8):
                vproj(j)

            # ---- attention for one (q-chunk, head-pair) ----
            A = [ap_pool.tile([P, S], BF16, tag=f"a{m}", bufs=1, name=f"a{m}")
                 for m in range(HD // P)]

            def attn(qc, hp):
                h0, h1 = 2 * hp, 2 * hp + 1
                nj = 4 * qc + 4
                av0 = psV.tile([DH + 1, SC], F32, tag="psV", name="av0")
                av1 = psV.tile([DH + 1, SC], F32, tag="psV", name="av1")
                ps_l = [None] * nj
                at_l = [None] * nj

                def scores(j):
                    d = j - 4 * qc
                    q0 = P * d if d >= 0 else 0
                    ps = psS.tile([P, 2 * SC], F32, tag="psS", name="ps")
                    if q0 > 0:
                        nc.vector.tensor_copy(ps[:, SC:SC + q0], negt[:, 0:q0])
                    nc.tensor.matmul(
                        ps[:, q0:SC],
                        (KT[hp][0:DH, P * j:P * (j + 1)]),
                        (QT[hp][0:DH, SC * qc + q0:SC * (qc + 1)]),
                        start=True, stop=True,
                    )
                    nc.tensor.matmul(
                        ps[:, SC + q0:2 * SC],
                        (KT[hp][DH:P, P * j:P * (j + 1)]),
                        (QT[hp][DH:P, SC * qc + q0:SC * (qc + 1)]),
                        start=True, stop=True,
                    )
                    at2 = at2p.tile([P, 2 * SC], BF16, tag="at2", name="at2")
                    nc.scalar.activation(at2[:, q0:2 * SC], ps[:, q0:2 * SC],
                                         EXP, scale=float(SCALE))
                    ps_l[j] = ps
                    at_l[j] = at2

                def av(j):
                    d = j - 4 * qc
                    q0 = P * d if d >= 0 else 0
                    at2 = at_l[j]
                    if d >= 0:
                        at3 = at2[:].rearrange("p (g c) -> p g c", g=2)
                        nc.gpsimd.tensor_tensor(
                            at3[:, 0, q0:q0 + P], at3[:, 0, q0:q0 + P],
                            tri[:], MULT)
                        nc.gpsimd.tensor_tensor(
                            at3[:, 1, q0:q0 + P], at3[:, 1, q0:q0 + P],
                            tri[:], MULT)
                    nc.tensor.matmul(
                        av0[:, q0:SC],
                        (Vg[j][:, (DH + 1) * h0:(DH + 1) * (h0 + 1)]),
                        (at2[:, q0:SC]),
                        start=(j == 0), stop=(j == nj - 1),
                    )
                    nc.tensor.matmul(
                        av1[:, q0:SC],
                        (Vg[j][:, (DH + 1) * h1:(DH + 1) * (h1 + 1)]),
                        (at2[:, SC + q0:2 * SC]),
                        start=(j == 0), stop=(j == nj - 1),
                    )
                    at_l[j] = None
                    ps_l[j] = None

                for j in range(nj):
                    scores(j)
                    if j > 0:
                        av(j - 1)
                av(nj - 1)

                for hh, avv in ((0, av0), (1, av1)):
                    rr = small.tile([1, SC], F32R, tag="rr", bufs=4, name="rr")
                    with nc.allow_low_precision(reason="f32r matmul feed"):
                        nc.vector.reciprocal(rr[:], avv[DH:DH + 1, :])
                    bc = psA.tile([DH, SC], F32, tag="psA", name="bc")
                    nc.tensor.matmul(bc[:], (ones64[:]), (rr[:]),
                                     start=True, stop=True)
                    rb = small.tile([DH, SC], F32, tag="rb", bufs=4, name="rb")
                    nc.vector.tensor_copy(rb[:], bc[:])
                    nc.vector.tensor_tensor(
                        A[hp][DH * hh:DH * (hh + 1), SC * qc:SC * (qc + 1)],
                        avv[0:DH, :], rb[:], MULT)

            # ---- output projection for one q-chunk ----
            def woproj(qc):
                for m in range(D // P):
                    ps = psA.tile([P, SC], F32, tag="psA")
                    for k in range(HD // P):
                        nc.tensor.matmul(
                            ps[:],
                            (wo_t[k][:, P * m:P * (m + 1)]),
                            (A[k][:, SC * qc:SC * (qc + 1)]),
                            start=(k == 0), stop=(k == HD // P - 1),
                        )
                    ot = otp.tile([P, SC], BF16, tag="ot")
                    nc.scalar.copy(ot[:], ps[:])
                    nc.sync.dma_start(
                        out_d[P * m:P * (m + 1), SC * qc:SC * (qc + 1)], ot[:])

            # ---- main schedule ----
            for qc in range(NSC):
                for hp in range(HD // P):
                    attn(qc, hp)
                    if qc == 0 and hp < 2:
                        for j in range(8 + 4 * hp, 12 + 4 * hp):
                            vproj(j)
                woproj(qc)

    nc.compile()
    return nc


_NC_CACHE = []


def _get_nc():
    if not _NC_CACHE:
        _NC_CACHE.append(build_program())
    return _NC_CACHE[0]


def _host_tables(token_positions):
    """cos/sin tables [128, S] matching the 16|16 quadrant row layout."""
    pos = np.asarray(token_positions).astype(np.float32)
    inv_freq = np.float32(THETA) ** (
        -np.arange(0, DH, 2, dtype=np.float32) / np.float32(DH))  # [32]
    ang = pos[:, None] * inv_freq[None, :]                # [S, 32]
    cos_t = np.cos(ang).T.astype(np.float32)              # [32, S]
    sin_t = np.sin(ang).T.astype(np.float32)
    # quadrant q (of 4): freqs 16*(q%2) .. +16, rows [c|c] / [-s|+s]
    crows, srows = [], []
    for q in range(4):
        f = slice(16 * (q % 2), 16 * (q % 2) + 16)
        crows += [cos_t[f], cos_t[f]]
        srows += [-sin_t[f], sin_t[f]]
    return np.concatenate(crows, 0), np.concatenate(srows, 0)


def _perm():
    """Per-head-pair row permutation: 16 even dims | 16 odd dims per
    32-row quadrant (so the RoPE pair-swap is intra-quadrant)."""
    perm1 = []
    for q in range(2):  # two quadrants per head
        perm1 += [2 * (16 * q + i) for i in range(16)]
        perm1 += [2 * (16 * q + i) + 1 for i in range(16)]
    perm1 = np.array(perm1)
    return np.concatenate([h * DH + perm1 for h in range(NH)])


def build_in_maps(in_features, token_positions, wq, wk, wv, wo):
    x = np.asarray(in_features, dtype=np.float32)
    wq = np.asarray(wq, dtype=np.float32)
    wk = np.asarray(wk, dtype=np.float32)
    wv = np.asarray(wv, dtype=np.float32)
    wo = np.asarray(wo, dtype=np.float32)

    cfull, sfull = _host_tables(token_positions)
    tri = np.triu(np.ones((P, P), dtype=np.float32))   # keep k_row <= q_col
    perm = _perm()
    bf = np.dtype("bfloat16") if hasattr(np, "bfloat16") else None

    def b16(a):
        import ml_dtypes
        return np.ascontiguousarray(a).astype(ml_dtypes.bfloat16)

    in_maps = []
    for c in range(NCORES):
        b, hg = divmod(c, HG)
        sl = slice(hg * HD, (hg + 1) * HD)
        in_maps.append({
            "xt": b16(x[b].T),
            "wqt": b16(wq[sl][perm].T),
            "wkt": b16(wk[sl][perm].T),
            "wvt": b16(wv[sl].T),
            "wot": b16(wo[:, sl].T),
            "cfull": b16(cfull),
            "sfull": b16(sfull),
            "tri": b16(tri),
            "onesr": np.ones((1, DH), dtype=np.float32),
            "onesb": b16(np.ones((P, NH), dtype=np.float32)),
            "negc": b16(np.full((P, 3 * P), -1e30, dtype=np.float32)),
        })
    return in_maps


def kernel(in_features, token_positions, wq, wk, wv, wo):
    _install_ntff_hook()
    in_maps = build_in_maps(in_features, token_positions, wq, wk, wv, wo)
    nc = _get_nc()
    res = run_bass_kernel_spmd(nc, in_maps, list(range(NCORES)))

    out = np.empty((B, S, D), dtype=np.float32)
    for b in range(B):
        acc = (np.asarray(res.results[2 * b]["outT"]).astype(np.float32)
               + np.asarray(res.results[2 * b + 1]["outT"]).astype(np.float32))
        out[b] = acc.T
    return out


# revision 15
# speedup vs baseline: 1.4923x; 1.0744x over previous
"""Multi-head attention with RoPE on 8 Trainium2 NeuronCores.

Sharding: core c handles batch b = c//2 and head-group hg = c%2 (8 of 16
heads).  Data-parallel over batch, tensor-parallel over heads; the
row-parallel wo all-reduce (2 cores per batch) happens on the host during
the gather/unshard step.

v2 per-core program (single SPMD NEFF, no collectives):
  - bf16 storage for x^T, weights, Q/K/V, attention weights and output
    (psum accumulation stays f32); halves DMA and SBUF traffic.
  - Projections: QT/KT = w @ x^T with 8-step psum accumulation; V in
    [s, 8*(64+1)] layout with a ones column per head (softmax denominator
    rides the attn@V matmul for free).
  - RoPE: host permutes wq/wk rows per head so the pair-swap is a
    16<->16 swap inside each 32-partition quadrant -> one DVE
    stream_shuffle (no SBUF<->SBUF DMA), then mul/mul/add with
    precomputed cos/sin tables (DVE + gpsimd).
  - Attention per (head-pair, q-chunk): for each 128-k-block j, the two
    heads' scores^T go into one 2-bank psum tile [128, 1024] via two
    row-tiled (64-contraction) matmuls that run concurrently on the PE;
    one exp ACTIVATE covers both heads (halves ACT instruction
    overhead); causal tri-mask on gpsimd; attn@V accumulates [65, 512]
    per head in psum.  Normalize via reciprocal (DVE) + ones-outer
    broadcast matmul (PE) + multiply (DVE).
  - wo projection per q-chunk is interleaved right after each q-chunk's
    attention so output DMA overlaps the next chunk's compute.
"""

import sys
import types

sys.path.insert(0, "/opt/trn_rl_repo")

import numpy as np

import concourse.bacc as bacc
import concourse.mybir as mybir
import concourse.tile as tile
from concourse.bass_utils import run_bass_kernel_spmd

# Problem constants (hardcoded per contract)
B, S, D = 4, 2048, 1024
H = 16
DH = D // H          # 64
THETA = 10000.0
NCORES = 8
HG = 2               # head groups (tensor-parallel factor)
HD = D // HG         # 512 = per-core heads dim
NH = H // HG         # 8 heads per core
P = 128
SC = 512             # q-chunk
NSC = S // SC        # 4
NKB = S // P         # 16 k-blocks
NDB = D // P         # 8 d-blocks (contraction for projections)
SCALE = 1.0 / np.sqrt(np.float32(DH))

F32 = mybir.dt.float32
F32R = mybir.dt.float32r
BF16 = mybir.dt.bfloat16

# stream_shuffle: swap halves within each 32-partition quadrant
SHUF_MASK = list(range(16, 32)) + list(range(16))


def _install_ntff_hook():
    """Best-effort: register the axon NTFF profile hook so trace=True works."""
    try:
        import antenv

        if "antenv.axon_hooks" in sys.modules:
            return
        sys.path.insert(0, "/root/.axon_site/trn_agent_boot")
        import trn_boot

        hook = trn_boot._ntff_profile_via_ctypes("/opt/axon/libaxon_pjrt.so")
        mod = types.ModuleType("antenv.axon_hooks")
        mod.get_axon_ntff_profile_hook = lambda: hook
        mod.set_axon_ntff_profile_hook = lambda h: None
        sys.modules["antenv.axon_hooks"] = mod
        antenv.axon_hooks = mod
    except Exception:
        pass


def build_program():
    nc = bacc.Bacc("TRN2", target_bir_lowering=False, debug=False,
                   num_devices=NCORES)

    xt_d = nc.dram_tensor("xt", [D, S], BF16, kind="ExternalInput")
    wqt_d = nc.dram_tensor("wqt", [D, HD], BF16, kind="ExternalInput")
    wkt_d = nc.dram_tensor("wkt", [D, HD], BF16, kind="ExternalInput")
    wvt_d = nc.dram_tensor("wvt", [D, HD], BF16, kind="ExternalInput")
    wot_d = nc.dram_tensor("wot", [HD, D], BF16, kind="ExternalInput")
    cf_d = nc.dram_tensor("cfull", [P, S], BF16, kind="ExternalInput")
    sf_d = nc.dram_tensor("sfull", [P, S], BF16, kind="ExternalInput")
    tri_d = nc.dram_tensor("tri", [P, P], BF16, kind="ExternalInput")
    onesr_d = nc.dram_tensor("onesr", [1, DH], F32R, kind="ExternalInput")
    onesb_d = nc.dram_tensor("onesb", [P, NH], BF16, kind="ExternalInput")
    negc_d = nc.dram_tensor("negc", [P, 3 * P], BF16, kind="ExternalInput")
    out_d = nc.dram_tensor("outT", [D, S], BF16, kind="ExternalOutput")

    EXP = mybir.ActivationFunctionType.Exp
    MULT = mybir.AluOpType.mult
    ADD = mybir.AluOpType.add

    with tile.TileContext(nc) as tc:
        with (
            tc.tile_pool(name="xtp", bufs=NDB) as xtp,
            tc.tile_pool(name="qk", bufs=8) as qk,
            tc.tile_pool(name="vg", bufs=NKB) as vgp,
            tc.tile_pool(name="w", bufs=8) as wp,
            tc.tile_pool(name="wo", bufs=4) as wop,
            tc.tile_pool(name="ap", bufs=4) as ap_pool,
            tc.tile_pool(name="at2", bufs=4) as at2p,
            tc.tile_pool(name="sw", bufs=2) as swp,
            tc.tile_pool(name="small", bufs=8) as small,
            tc.tile_pool(name="ot", bufs=4) as otp,
            tc.tile_pool(name="psA", bufs=2, space="PSUM") as psA,
            tc.tile_pool(name="psS", bufs=2, space="PSUM") as psS,
            tc.tile_pool(name="psV", bufs=2, space="PSUM") as psV,
        ):
            # ---- constants / DMAs (weights first so Q-proj starts early) ----
            ones64 = small.tile([1, DH], F32R, tag="ones64", bufs=1)
            nc.sync.dma_start(ones64[:], onesr_d[:])
            ones_sb = small.tile([P, NH], BF16, tag="onesb", bufs=1)
            nc.sync.dma_start(ones_sb[:], onesb_d[:])
            negt = small.tile([P, 3 * P], BF16, tag="negc", bufs=1)
            nc.sync.dma_start(negt[:], negc_d[:])

            wq_t = []
            for k in range(NDB):
                t = wp.tile([P, HD], BF16, tag="w", name=f"wq{k}")
                nc.sync.dma_start(t[:], wqt_d[P * k:P * (k + 1), :])
                wq_t.append(t)
            xt = []
            for k in range(NDB):
                t = xtp.tile([P, S], BF16, tag="xt", name=f"xt{k}")
                nc.sync.dma_start(t[:], xt_d[P * k:P * (k + 1), :])
                xt.append(t)
            wk_t = []
            for k in range(NDB):
                t = wp.tile([P, HD], BF16, tag="w", name=f"wk{k}")
                nc.sync.dma_start(t[:], wkt_d[P * k:P * (k + 1), :])
                wk_t.append(t)
            wv_t = []
            for k in range(NDB):
                t = wp.tile([P, HD], BF16, tag="w", name=f"wv{k}")
                nc.sync.dma_start(t[:], wvt_d[P * k:P * (k + 1), :])
                wv_t.append(t)
            cf = small.tile([P, S], BF16, tag="cf", bufs=1)
            nc.sync.dma_start(cf[:], cf_d[:])
            sf = small.tile([P, S], BF16, tag="sf", bufs=1)
            nc.sync.dma_start(sf[:], sf_d[:])
            tri = small.tile([P, P], BF16, tag="tri", bufs=1)
            nc.sync.dma_start(tri[:], tri_d[:])
            wo_t = []
            for k in range(HD // P):
                t = wop.tile([P, D], BF16, tag="wot", name=f"wo{k}")
                nc.sync.dma_start(t[:], wot_d[P * k:P * (k + 1), :])
                wo_t.append(t)

            # ---- Q/K projections + RoPE ----
            def rope(t):
                sw = swp.tile([P, S], BF16, tag="sw", name="sw")
                nc.vector.stream_shuffle(sw[:], t[:], SHUF_MASK)
                nc.gpsimd.tensor_tensor(sw[:], sw[:], sf[:], MULT)
                nc.vector.tensor_tensor(t[:], t[:], cf[:], MULT)
                nc.vector.tensor_tensor(t[:], t[:], sw[:], ADD)

            def project_T(wt, out_tiles, do_rope):
                for m in range(HD // P):
                    for n in range(NSC):
                        ps = psA.tile([P, SC], F32, tag="psA")
                        for k in range(NDB):
                            nc.tensor.matmul(
                                ps[:],
                                (wt[k][:, P * m:P * (m + 1)]),
                                (xt[k][:, SC * n:SC * (n + 1)]),
                                start=(k == 0), stop=(k == NDB - 1),
                            )
                        nc.vector.tensor_copy(
                            out_tiles[m][:, SC * n:SC * (n + 1)], ps[:])
                    if do_rope:
                        rope(out_tiles[m])

            QT = [qk.tile([P, S], BF16, tag=f"qt{m}", bufs=1, name=f"qt{m}")
                  for m in range(HD // P)]
            project_T(wq_t, QT, True)
            KT = [qk.tile([P, S], BF16, tag=f"kt{m}", bufs=1, name=f"kt{m}")
                  for m in range(HD // P)]
            project_T(wk_t, KT, True)

            # ---- V projection (j = k-block); interleaved with attention ----
            Vg = [None] * NKB

            def vproj(j):
                vt = vgp.tile([P, NH * (DH + 1)], BF16, tag=f"vg{j}", bufs=1,
                              name=f"vg{j}")
                v3 = vt[:].rearrange("p (h c) -> p h c", h=NH)
                ps = psA.tile([P, HD], F32, tag="psA")
                for k in range(NDB):
                    nc.tensor.matmul(
                        ps[:],
                        (xt[k][:, P * j:P * (j + 1)]),
                        (wv_t[k][:]),
                        start=(k == 0), stop=(k == NDB - 1),
                    )
                nc.vector.tensor_copy(
                    v3[:, :, 0:DH], ps[:].rearrange("p (h c) -> p h c", h=NH))
                nc.vector.tensor_copy(v3[:, :, DH:DH + 1], ones_sb[:, :, None])
                Vg[j] = vt

            for j in range(8):
                vproj(j)

            # ---- attention for one (q-chunk, head-pair) ----
            A = [ap_pool.tile([P, S], BF16, tag=f"a{m}", bufs=1, name=f"a{m}")
                 for m in range(HD // P)]

            def attn(qc, hp):
                h0, h1 = 2 * hp, 2 * hp + 1
                nj = 4 * qc + 4
                av0 = psV.tile([DH + 1, SC], F32, tag="psV", name="av0")
                av1 = psV.tile([DH + 1, SC], F32, tag="psV", name="av1")
                ps_l = [None] * nj
                at_l = [None] * nj

                def scores(j):
                    d = j - 4 * qc
                    q0 = P * d if d >= 0 else 0
                    ps = psS.tile([P, 2 * SC], F32, tag="psS", name="ps")
                    if q0 > 0:
                        nc.vector.tensor_copy(ps[:, SC:SC + q0], negt[:, 0:q0])
                    nc.tensor.matmul(
                        ps[:, q0:SC],
                        (KT[hp][0:DH, P * j:P * (j + 1)]),
                        (QT[hp][0:DH, SC * qc + q0:SC * (qc + 1)]),
                        start=True, stop=True,
                    )
                    nc.tensor.matmul(
                        ps[:, SC + q0:2 * SC],
                        (KT[hp][DH:P, P * j:P * (j + 1)]),
                        (QT[hp][DH:P, SC * qc + q0:SC * (qc + 1)]),
                        start=True, stop=True,
                    )
                    at2 = at2p.tile([P, 2 * SC], BF16, tag="at2", name="at2")
                    nc.scalar.activation(at2[:, q0:2 * SC], ps[:, q0:2 * SC],
                                         EXP, scale=float(SCALE))
                    ps_l[j] = ps
                    at_l[j] = at2

                def av(j):
                    d = j - 4 * qc
                    q0 = P * d if d >= 0 else 0
                    at2 = at_l[j]
                    if d >= 0:
                        at3 = at2[:].rearrange("p (g c) -> p g c", g=2)
                        nc.vector.tensor_tensor(
                            at3[:, 0, q0:q0 + P], at3[:, 0, q0:q0 + P],
                            tri[:], MULT)
                        nc.vector.tensor_tensor(
                            at3[:, 1, q0:q0 + P], at3[:, 1, q0:q0 + P],
                            tri[:], MULT)
                    nc.tensor.matmul(
                        av0[:, q0:SC],
                        (Vg[j][:, (DH + 1) * h0:(DH + 1) * (h0 + 1)]),
                        (at2[:, q0:SC]),
                        start=(j == 0), stop=(j == nj - 1),
                    )
                    nc.tensor.matmul(
                        av1[:, q0:SC],
                        (Vg[j][:, (DH + 1) * h1:(DH + 1) * (h1 + 1)]),
                        (at2[:, SC + q0:2 * SC]),
                        start=(j == 0), stop=(j == nj - 1),
                    )
                    at_l[j] = None
                    ps_l[j] = None

                for j in range(nj):
                    scores(j)
                    if j > 0:
                        av(j - 1)
                av(nj - 1)

                LOG = mybir.ActivationFunctionType.Ln
                for hh, avv in ((0, av0), (1, av1)):
                    # 1/denom = exp(-log(denom)) on ACT (2-ULP tables);
                    # copy av out of psum early so the bank frees fast.
                    avr = small.tile([DH, SC], BF16, tag="avr", bufs=4,
                                     name="avr")
                    nc.vector.tensor_copy(avr[:], avv[0:DH, :])
                    ld = small.tile([1, SC], F32, tag="ld", bufs=4, name="ld")
                    nc.scalar.activation(ld[:], avv[DH:DH + 1, :], LOG)
                    rr = small.tile([1, SC], F32R, tag="rr", bufs=4, name="rr")
                    with nc.allow_low_precision(reason="f32r matmul feed"):
                        nc.scalar.activation(rr[:], ld[:], EXP, scale=-1.0)
                    bc = psA.tile([DH, SC], F32, tag="psA", name="bc")
                    nc.tensor.matmul(bc[:], (ones64[:]), (rr[:]),
                                     start=True, stop=True)
                    rb = small.tile([DH, SC], F32, tag="rb", bufs=4, name="rb")
                    nc.vector.tensor_copy(rb[:], bc[:])
                    nc.vector.tensor_tensor(
                        A[hp][DH * hh:DH * (hh + 1), SC * qc:SC * (qc + 1)],
                        avr[:], rb[:], MULT)

            # ---- output projection for one q-chunk ----
            def woproj(qc):
                for m in range(D // P):
                    ps = psA.tile([P, SC], F32, tag="psA")
                    for k in range(HD // P):
                        nc.tensor.matmul(
                            ps[:],
                            (wo_t[k][:, P * m:P * (m + 1)]),
                            (A[k][:, SC * qc:SC * (qc + 1)]),
                            start=(k == 0), stop=(k == HD // P - 1),
                        )
                    ot = otp.tile([P, SC], BF16, tag="ot")
                    nc.vector.tensor_copy(ot[:], ps[:])
                    nc.sync.dma_start(
                        out_d[P * m:P * (m + 1), SC * qc:SC * (qc + 1)], ot[:])

            # ---- main schedule ----
            for qc in range(NSC):
                for hp in range(HD // P):
                    attn(qc, hp)
                    if qc == 0 and hp < 2:
                        for j in range(8 + 4 * hp, 12 + 4 * hp):
                            vproj(j)
                woproj(qc)

    nc.compile()
    return nc


_NC_CACHE = []


def _get_nc():
    if not _NC_CACHE:
        _NC_CACHE.append(build_program())
    return _NC_CACHE[0]


def _host_tables(token_positions):
    """cos/sin tables [128, S] matching the 16|16 quadrant row layout."""
    pos = np.asarray(token_positions).astype(np.float32)
    inv_freq = np.float32(THETA) ** (
        -np.arange(0, DH, 2, dtype=np.float32) / np.float32(DH))  # [32]
    ang = pos[:, None] * inv_freq[None, :]                # [S, 32]
    cos_t = np.cos(ang).T.astype(np.float32)              # [32, S]
    sin_t = np.sin(ang).T.astype(np.float32)
    # quadrant q (of 4): freqs 16*(q%2) .. +16, rows [c|c] / [-s|+s]
    crows, srows = [], []
    for q in range(4):
        f = slice(16 * (q % 2), 16 * (q % 2) + 16)
        crows += [cos_t[f], cos_t[f]]
        srows += [-sin_t[f], sin_t[f]]
    return np.concatenate(crows, 0), np.concatenate(srows, 0)


def _perm():
    """Per-head-pair row permutation: 16 even dims | 16 odd dims per
    32-row quadrant (so the RoPE pair-swap is intra-quadrant)."""
    perm1 = []
    for q in range(2):  # two quadrants per head
        perm1 += [2 * (16 * q + i) for i in range(16)]
        perm1 += [2 * (16 * q + i) + 1 for i in range(16)]
    perm1 = np.array(perm1)
    return np.concatenate([h * DH + perm1 for h in range(NH)])


def build_in_maps(in_features, token_positions, wq, wk, wv, wo):
    x = np.asarray(in_features, dtype=np.float32)
    wq = np.asarray(wq, dtype=np.float32)
    wk = np.asarray(wk, dtype=np.float32)
    wv = np.asarray(wv, dtype=np.float32)
    wo = np.asarray(wo, dtype=np.float32)

    cfull, sfull = _host_tables(token_positions)
    tri = np.triu(np.ones((P, P), dtype=np.float32))   # keep k_row <= q_col
    perm = _perm()
    bf = np.dtype("bfloat16") if hasattr(np, "bfloat16") else None

    def b16(a):
        import ml_dtypes
        return np.ascontiguousarray(a).astype(ml_dtypes.bfloat16)

    in_maps = []
    for c in range(NCORES):
        b, hg = divmod(c, HG)
        sl = slice(hg * HD, (hg + 1) * HD)
        in_maps.append({
            "xt": b16(x[b].T),
            "wqt": b16(wq[sl][perm].T),
            "wkt": b16(wk[sl][perm].T),
            "wvt": b16(wv[sl].T),
            "wot": b16(wo[:, sl].T),
            "cfull": b16(cfull),
            "sfull": b16(sfull),
            "tri": b16(tri),
            "onesr": np.ones((1, DH), dtype=np.float32),
            "onesb": b16(np.ones((P, NH), dtype=np.float32)),
            "negc": b16(np.full((P, 3 * P), -1e30, dtype=np.float32)),
        })
    return in_maps


def kernel(in_features, token_positions, wq, wk, wv, wo):
    _install_ntff_hook()
    in_maps = build_in_maps(in_features, token_positions, wq, wk, wv, wo)
    nc = _get_nc()
    res = run_bass_kernel_spmd(nc, in_maps, list(range(NCORES)))

    out = np.empty((B, S, D), dtype=np.float32)
    for b in range(B):
        acc = (np.asarray(res.results[2 * b]["outT"]).astype(np.float32)
               + np.asarray(res.results[2 * b + 1]["outT"]).astype(np.float32))
        out[b] = acc.T
    return out


# revision 21
# speedup vs baseline: 1.4941x; 1.0013x over previous
"""Multi-head attention with RoPE on 8 Trainium2 NeuronCores.

Sharding: core c handles batch b = c//2 and head-group hg = c%2 (8 of 16
heads).  Data-parallel over batch, tensor-parallel over heads; the
row-parallel wo all-reduce (2 cores per batch) happens on the host during
the gather/unshard step.

v2 per-core program (single SPMD NEFF, no collectives):
  - bf16 storage for x^T, weights, Q/K/V, attention weights and output
    (psum accumulation stays f32); halves DMA and SBUF traffic.
  - Projections: QT/KT = w @ x^T with 8-step psum accumulation; V in
    [s, 8*(64+1)] layout with a ones column per head (softmax denominator
    rides the attn@V matmul for free).
  - RoPE: host permutes wq/wk rows per head so the pair-swap is a
    16<->16 swap inside each 32-partition quadrant -> one DVE
    stream_shuffle (no SBUF<->SBUF DMA), then mul/mul/add with
    precomputed cos/sin tables (DVE + gpsimd).
  - Attention per (head-pair, q-chunk): for each 128-k-block j, the two
    heads' scores^T go into one 2-bank psum tile [128, 1024] via two
    row-tiled (64-contraction) matmuls that run concurrently on the PE;
    one exp ACTIVATE covers both heads (halves ACT instruction
    overhead); causal tri-mask on gpsimd; attn@V accumulates [65, 512]
    per head in psum.  Normalize via reciprocal (DVE) + ones-outer
    broadcast matmul (PE) + multiply (DVE).
  - wo projection per q-chunk is interleaved right after each q-chunk's
    attention so output DMA overlaps the next chunk's compute.
"""

import sys
import types

sys.path.insert(0, "/opt/trn_rl_repo")

import numpy as np

import concourse.bacc as bacc
import concourse.mybir as mybir
import concourse.tile as tile
from concourse.bass_utils import run_bass_kernel_spmd

# Problem constants (hardcoded per contract)
B, S, D = 4, 2048, 1024
H = 16
DH = D // H          # 64
THETA = 10000.0
NCORES = 8
HG = 2               # head groups (tensor-parallel factor)
HD = D // HG         # 512 = per-core heads dim
NH = H // HG         # 8 heads per core
P = 128
SC = 512             # q-chunk
NSC = S // SC        # 4
NKB = S // P         # 16 k-blocks
NDB = D // P         # 8 d-blocks (contraction for projections)
SCALE = 1.0 / np.sqrt(np.float32(DH))

F32 = mybir.dt.float32
F32R = mybir.dt.float32r
BF16 = mybir.dt.bfloat16

# stream_shuffle: swap halves within each 32-partition quadrant
SHUF_MASK = list(range(16, 32)) + list(range(16))


def _install_ntff_hook():
    """Best-effort: register the axon NTFF profile hook so trace=True works."""
    try:
        import antenv

        if "antenv.axon_hooks" in sys.modules:
            return
        sys.path.insert(0, "/root/.axon_site/trn_agent_boot")
        import trn_boot

        hook = trn_boot._ntff_profile_via_ctypes("/opt/axon/libaxon_pjrt.so")
        mod = types.ModuleType("antenv.axon_hooks")
        mod.get_axon_ntff_profile_hook = lambda: hook
        mod.set_axon_ntff_profile_hook = lambda h: None
        sys.modules["antenv.axon_hooks"] = mod
        antenv.axon_hooks = mod
    except Exception:
        pass


def build_program():
    nc = bacc.Bacc("TRN2", target_bir_lowering=False, debug=False,
                   num_devices=NCORES)

    xt_d = nc.dram_tensor("xt", [D, S], BF16, kind="ExternalInput")
    wqt_d = nc.dram_tensor("wqt", [D, HD], BF16, kind="ExternalInput")
    wkt_d = nc.dram_tensor("wkt", [D, HD], BF16, kind="ExternalInput")
    wvt_d = nc.dram_tensor("wvt", [D, HD], BF16, kind="ExternalInput")
    wot_d = nc.dram_tensor("wot", [HD, D], BF16, kind="ExternalInput")
    cf_d = nc.dram_tensor("cfull", [P, S], BF16, kind="ExternalInput")
    sf_d = nc.dram_tensor("sfull", [P, S], BF16, kind="ExternalInput")
    tri_d = nc.dram_tensor("tri", [P, P], BF16, kind="ExternalInput")
    onesr_d = nc.dram_tensor("onesr", [1, DH], F32R, kind="ExternalInput")
    onesb_d = nc.dram_tensor("onesb", [P, NH], BF16, kind="ExternalInput")
    negc_d = nc.dram_tensor("negc", [P, 3 * P], BF16, kind="ExternalInput")
    out_d = nc.dram_tensor("outT", [D, S], BF16, kind="ExternalOutput")

    EXP = mybir.ActivationFunctionType.Exp
    MULT = mybir.AluOpType.mult
    ADD = mybir.AluOpType.add

    with tile.TileContext(nc) as tc:
        with (
            tc.tile_pool(name="xtp", bufs=NDB) as xtp,
            tc.tile_pool(name="qk", bufs=8) as qk,
            tc.tile_pool(name="vg", bufs=NKB) as vgp,
            tc.tile_pool(name="w", bufs=24) as wp,
            tc.tile_pool(name="wo", bufs=4) as wop,
            tc.tile_pool(name="ap", bufs=4) as ap_pool,
            tc.tile_pool(name="at2", bufs=4) as at2p,
            tc.tile_pool(name="sw", bufs=2) as swp,
            tc.tile_pool(name="small", bufs=8) as small,
            tc.tile_pool(name="ot", bufs=4) as otp,
            tc.tile_pool(name="psA", bufs=2, space="PSUM") as psA,
            tc.tile_pool(name="psS", bufs=2, space="PSUM") as psS,
            tc.tile_pool(name="psV", bufs=1, space="PSUM") as psV,
        ):
            # ---- constants / DMAs (weights first so Q-proj starts early) ----
            ones64 = small.tile([1, DH], F32R, tag="ones64", bufs=1)
            nc.sync.dma_start(ones64[:], onesr_d[:])
            ones_sb = small.tile([P, NH], BF16, tag="onesb", bufs=1)
            nc.sync.dma_start(ones_sb[:], onesb_d[:])
            negt = small.tile([P, 3 * P], BF16, tag="negc", bufs=1)
            nc.sync.dma_start(negt[:], negc_d[:])

            wq_t = []
            for k in range(NDB):
                t = wp.tile([P, HD], BF16, tag="w", name=f"wq{k}")
                nc.sync.dma_start(t[:], wqt_d[P * k:P * (k + 1), :])
                wq_t.append(t)
            xt = []
            for k in range(NDB):
                t = xtp.tile([P, S], BF16, tag="xt", name=f"xt{k}")
                nc.sync.dma_start(t[:], xt_d[P * k:P * (k + 1), :])
                xt.append(t)
            wk_t = []
            for k in range(NDB):
                t = wp.tile([P, HD], BF16, tag="w", name=f"wk{k}")
                nc.sync.dma_start(t[:], wkt_d[P * k:P * (k + 1), :])
                wk_t.append(t)
            wv_t = []
            for k in range(NDB):
                t = wp.tile([P, HD], BF16, tag="w", name=f"wv{k}")
                nc.sync.dma_start(t[:], wvt_d[P * k:P * (k + 1), :])
                wv_t.append(t)
            cf = small.tile([P, S], BF16, tag="cf", bufs=1)
            nc.sync.dma_start(cf[:], cf_d[:])
            sf = small.tile([P, S], BF16, tag="sf", bufs=1)
            nc.sync.dma_start(sf[:], sf_d[:])
            tri = small.tile([P, P], BF16, tag="tri", bufs=1)
            nc.sync.dma_start(tri[:], tri_d[:])
            wo_t = []
            for k in range(HD // P):
                t = wop.tile([P, D], BF16, tag="wot", name=f"wo{k}")
                nc.sync.dma_start(t[:], wot_d[P * k:P * (k + 1), :])
                wo_t.append(t)

            # ---- Q/K projections + RoPE, one q-chunk column slice at a time
            # (lets chunk qc+1's projection PE work overlap chunk qc's
            # ACT-heavy attention) ----
            def rope_slice(t, n):
                cs = slice(SC * n, SC * (n + 1))
                sw = swp.tile([P, SC], BF16, tag="sw", name="sw")
                nc.vector.stream_shuffle(sw[:], t[:, cs], SHUF_MASK)
                nc.gpsimd.tensor_tensor(sw[:], sw[:], sf[:, cs], MULT)
                nc.vector.tensor_tensor(t[:, cs], t[:, cs], cf[:, cs], MULT)
                nc.vector.tensor_tensor(t[:, cs], t[:, cs], sw[:], ADD)

            def project_slice(wt, out_tiles, n):
                for m in range(HD // P):
                    ps = psA.tile([P, SC], F32, tag="psA")
                    for k in range(NDB):
                        nc.tensor.matmul(
                            ps[:],
                            (wt[k][:, P * m:P * (m + 1)]),
                            (xt[k][:, SC * n:SC * (n + 1)]),
                            start=(k == 0), stop=(k == NDB - 1),
                        )
                    nc.vector.tensor_copy(
                        out_tiles[m][:, SC * n:SC * (n + 1)], ps[:])
                    rope_slice(out_tiles[m], n)

            QT = [qk.tile([P, S], BF16, tag=f"qt{m}", bufs=1, name=f"qt{m}")
                  for m in range(HD // P)]
            KT = [qk.tile([P, S], BF16, tag=f"kt{m}", bufs=1, name=f"kt{m}")
                  for m in range(HD // P)]
            project_slice(wq_t, QT, 0)
            project_slice(wk_t, KT, 0)

            # ---- V projection (j = k-block); interleaved with attention ----
            Vg = [None] * NKB

            def vproj(j):
                vt = vgp.tile([P, NH * (DH + 1)], BF16, tag=f"vg{j}", bufs=1,
                              name=f"vg{j}")
                v3 = vt[:].rearrange("p (h c) -> p h c", h=NH)
                ps = psA.tile([P, HD], F32, tag="psA")
                for k in range(NDB):
                    nc.tensor.matmul(
                        ps[:],
                        (xt[k][:, P * j:P * (j + 1)]),
                        (wv_t[k][:]),
                        start=(k == 0), stop=(k == NDB - 1),
                    )
                nc.vector.tensor_copy(
                    v3[:, :, 0:DH], ps[:].rearrange("p (h c) -> p h c", h=NH))
                nc.vector.tensor_copy(v3[:, :, DH:DH + 1], ones_sb[:, :, None])
                Vg[j] = vt

            for j in range(4):
                vproj(j)

            # ---- attention for one (q-chunk, head-pair) ----
            A = [ap_pool.tile([P, S], BF16, tag=f"a{m}", bufs=1, name=f"a{m}")
                 for m in range(HD // P)]

            def attn(qc, hp):
                h0, h1 = 2 * hp, 2 * hp + 1
                nj = 4 * qc + 4
                av = psV.tile([DH + 1, 2 * SC], F32, tag="psV", name="av")
                ps_l = [None] * nj
                at_l = [None] * nj

                def scores(j):
                    d = j - 4 * qc
                    q0 = P * d if d >= 0 else 0
                    ps = psS.tile([P, 2 * SC], F32, tag="psS", name="ps")
                    if q0 > 0:
                        nc.vector.tensor_copy(ps[:, SC:SC + q0], negt[:, 0:q0])
                    nc.tensor.matmul(
                        ps[:, q0:SC],
                        (KT[hp][0:DH, P * j:P * (j + 1)]),
                        (QT[hp][0:DH, SC * qc + q0:SC * (qc + 1)]),
                        start=True, stop=True,
                    )
                    nc.tensor.matmul(
                        ps[:, SC + q0:2 * SC],
                        (KT[hp][DH:P, P * j:P * (j + 1)]),
                        (QT[hp][DH:P, SC * qc + q0:SC * (qc + 1)]),
                        start=True, stop=True,
                    )
                    at2 = at2p.tile([P, 2 * SC], BF16, tag="at2", name="at2")
                    nc.scalar.activation(at2[:, q0:2 * SC], ps[:, q0:2 * SC],
                                         EXP, scale=float(SCALE))
                    ps_l[j] = ps
                    at_l[j] = at2

                def do_av(j):
                    d = j - 4 * qc
                    q0 = P * d if d >= 0 else 0
                    at2 = at_l[j]
                    if d >= 0:
                        at3 = at2[:].rearrange("p (g c) -> p g c", g=2)
                        nc.vector.tensor_tensor(
                            at3[:, 0, q0:q0 + P], at3[:, 0, q0:q0 + P],
                            tri[:], MULT)
                        nc.vector.tensor_tensor(
                            at3[:, 1, q0:q0 + P], at3[:, 1, q0:q0 + P],
                            tri[:], MULT)
                    nc.tensor.matmul(
                        av[:, q0:SC],
                        (Vg[j][:, (DH + 1) * h0:(DH + 1) * (h0 + 1)]),
                        (at2[:, q0:SC]),
                        start=(j == 0), stop=(j == nj - 1),
                    )
                    nc.tensor.matmul(
                        av[:, SC + q0:2 * SC],
                        (Vg[j][:, (DH + 1) * h1:(DH + 1) * (h1 + 1)]),
                        (at2[:, SC + q0:2 * SC]),
                        start=(j == 0), stop=(j == nj - 1),
                    )
                    at_l[j] = None
                    ps_l[j] = None

                for j in range(nj):
                    scores(j)
                    if j > 0:
                        do_av(j - 1)
                do_av(nj - 1)

                # 1/denom = exp(-log(denom)) on ACT (2-ULP tables), both
                # heads in one pass; copy av out of psum early so the 2
                # banks free fast.
                LOG = mybir.ActivationFunctionType.Ln
                avr = small.tile([DH, 2 * SC], BF16, tag="avr", bufs=2,
                                 name="avr")
                nc.vector.tensor_copy(avr[:], av[0:DH, :])
                ld = small.tile([1, 2 * SC], F32, tag="ld", bufs=2, name="ld")
                nc.scalar.activation(ld[:], av[DH:DH + 1, :], LOG)
                rr = small.tile([1, 2 * SC], F32R, tag="rr", bufs=2, name="rr")
                with nc.allow_low_precision(reason="f32r matmul feed"):
                    nc.scalar.activation(rr[:], ld[:], EXP, scale=-1.0)
                for hh in (0, 1):
                    bc = psA.tile([DH, SC], F32, tag="psA", name="bc")
                    nc.tensor.matmul(bc[:], (ones64[:]),
                                     (rr[:, SC * hh:SC * (hh + 1)]),
                                     start=True, stop=True)
                    rb = small.tile([DH, SC], F32, tag="rb", bufs=4, name="rb")
                    nc.vector.tensor_copy(rb[:], bc[:])
                    nc.vector.tensor_tensor(
                        A[hp][DH * hh:DH * (hh + 1), SC * qc:SC * (qc + 1)],
                        avr[:, SC * hh:SC * (hh + 1)], rb[:], MULT)

            # ---- output projection for one q-chunk ----
            def woproj(qc):
                for m in range(D // P):
                    ps = psA.tile([P, SC], F32, tag="psA")
                    for k in range(HD // P):
                        nc.tensor.matmul(
                            ps[:],
                            (wo_t[k][:, P * m:P * (m + 1)]),
                            (A[k][:, SC * qc:SC * (qc + 1)]),
                            start=(k == 0), stop=(k == HD // P - 1),
                        )
                    ot = otp.tile([P, SC], BF16, tag="ot")
                    nc.vector.tensor_copy(ot[:], ps[:])
                    nc.sync.dma_start(
                        out_d[P * m:P * (m + 1), SC * qc:SC * (qc + 1)], ot[:])

            # ---- main schedule ----
            for qc in range(NSC):
                for hp in range(HD // P):
                    attn(qc, hp)
                    if qc < NSC - 1:
                        if hp == 0:
                            project_slice(wq_t, QT, qc + 1)
                        elif hp == 1:
                            project_slice(wk_t, KT, qc + 1)
                        elif hp == 2:
                            for j in range(4 * qc + 4, 4 * qc + 8):
                                vproj(j)
                woproj(qc)

    nc.compile()
    return nc


_NC_CACHE = []


def _get_nc():
    if not _NC_CACHE:
        _NC_CACHE.append(build_program())
    return _NC_CACHE[0]


def _host_tables(token_positions):
    """cos/sin tables [128, S] matching the 16|16 quadrant row layout."""
    pos = np.asarray(token_positions).astype(np.float32)
    inv_freq = np.float32(THETA) ** (
        -np.arange(0, DH, 2, dtype=np.float32) / np.float32(DH))  # [32]
    ang = pos[:, None] * inv_freq[None, :]                # [S, 32]
    cos_t = np.cos(ang).T.astype(np.float32)              # [32, S]
    sin_t = np.sin(ang).T.astype(np.float32)
    # quadrant q (of 4): freqs 16*(q%2) .. +16, rows [c|c] / [-s|+s]
    crows, srows = [], []
    for q in range(4):
        f = slice(16 * (q % 2), 16 * (q % 2) + 16)
        crows += [cos_t[f], cos_t[f]]
        srows += [-sin_t[f], sin_t[f]]
    return np.concatenate(crows, 0), np.concatenate(srows, 0)


def _perm():
    """Per-head-pair row permutation: 16 even dims | 16 odd dims per
    32-row quadrant (so the RoPE pair-swap is intra-quadrant)."""
    perm1 = []
    for q in range(2):  # two quadrants per head
        perm1 += [2 * (16 * q + i) for i in range(16)]
        perm1 += [2 * (16 * q + i) + 1 for i in range(16)]
    perm1 = np.array(perm1)
    return np.concatenate([h * DH + perm1 for h in range(NH)])


def build_in_maps(in_features, token_positions, wq, wk, wv, wo):
    x = np.asarray(in_features, dtype=np.float32)
    wq = np.asarray(wq, dtype=np.float32)
    wk = np.asarray(wk, dtype=np.float32)
    wv = np.asarray(wv, dtype=np.float32)
    wo = np.asarray(wo, dtype=np.float32)

    cfull, sfull = _host_tables(token_positions)
    tri = np.triu(np.ones((P, P), dtype=np.float32))   # keep k_row <= q_col
    perm = _perm()
    bf = np.dtype("bfloat16") if hasattr(np, "bfloat16") else None

    def b16(a):
        import ml_dtypes
        return np.ascontiguousarray(a).astype(ml_dtypes.bfloat16)

    in_maps = []
    for c in range(NCORES):
        b, hg = divmod(c, HG)
        sl = slice(hg * HD, (hg + 1) * HD)
        in_maps.append({
            "xt": b16(x[b].T),
            "wqt": b16(wq[sl][perm].T),
            "wkt": b16(wk[sl][perm].T),
            "wvt": b16(wv[sl].T),
            "wot": b16(wo[:, sl].T),
            "cfull": b16(cfull),
            "sfull": b16(sfull),
            "tri": b16(tri),
            "onesr": np.ones((1, DH), dtype=np.float32),
            "onesb": b16(np.ones((P, NH), dtype=np.float32)),
            "negc": b16(np.full((P, 3 * P), -1e30, dtype=np.float32)),
        })
    return in_maps


def kernel(in_features, token_positions, wq, wk, wv, wo):
    _install_ntff_hook()
    in_maps = build_in_maps(in_features, token_positions, wq, wk, wv, wo)
    nc = _get_nc()
    res = run_bass_kernel_spmd(nc, in_maps, list(range(NCORES)))

    out = np.empty((B, S, D), dtype=np.float32)
    for b in range(B):
        acc = (np.asarray(res.results[2 * b]["outT"]).astype(np.float32)
               + np.asarray(res.results[2 * b + 1]["outT"]).astype(np.float32))
        out[b] = acc.T
    return out
